# revision 12
# baseline (speedup 1.0000x reference)
"""Multi-head attention (B=2, S=2048, D=2048, H=16) on 8 TRN2 NeuronCores.

Tensor-parallel over heads: each core computes 2 of 16 heads end to end
(q/k/v projections column-sharded, out_proj row-sharded) and writes a
partial output; the host sums the 8 partials and adds the output bias.

Per-core device pipeline (b = batch index, looped):
  phase 1: q^T,k^T ([HD, S] layout) and v ([S, HD] layout) via matmuls
           against host-pre-transposed hidden^T; biases are added with a
           K=1 matmul against a ones row. k is PE-transposed to [S, HD]
           for the k output.
  phase 2: scores^T[s_k, s_q] = k^T.T @ q^T (1/sqrt(HD) folded into Wq),
           + mask^T (DVE), exp (ACT, bf16 out), then ctx[s_q, :] =
           exp(S)^T.T @ [V | 1] -- the appended ones column yields the
           softmax denominator in the same accumulation. Divide by it
           (per-partition scalar), PE-transpose ctx to [HD, s_q].
  phase 3: out_partial[s, m] = sum_h ctx^T_h.T @ Wo^T_h.

Matmuls for projections/scores/out_proj run in float32r (single-pass
fp32 on the PE at bf16 rate); the PV matmul runs in bf16 on softmax
weights in [0,1]. All accumulation is fp32 in PSUM.
"""

import os
import sys
import types

for _p in ("/opt/trn_rl_repo", "/root/.axon_site/_ro/trn_rl_repo", "/root/.axon_site"):
    if os.path.isdir(_p) and _p not in sys.path:
        sys.path.append(_p)

import numpy as np

import concourse.bass as bass
import concourse.mybir as mybir
import concourse.tile as tile
from concourse.bass_utils import run_bass_kernel_spmd

# problem shape (hardcoded per contest contract)
B, S, D, H, HD = 2, 2048, 2048, 16, 128
NCORES = 8
HPC = H // NCORES          # heads per core = 2
DSH = HPC * HD             # per-core model-dim shard = 256
P = 128                    # partitions
NCH = D // P               # contraction chunks = 16
NSB = S // P               # 128-row blocks per batch = 16
SQ = 256                   # strip width (free dim of matmuls)
NST = S // SQ              # strips per batch = 8

F32 = mybir.dt.float32
F32R = mybir.dt.float32r   # projection/scores/out_proj matmul dtype
PV_DT = mybir.dt.bfloat16  # exp(S) and V-augmented dtypes for the PV matmul
AF = mybir.ActivationFunctionType

MASK_CLAMP = -60.0


def _split_multiwait_insts(nc):
    """This walrus build rejects any instruction carrying more than one
    sync-wait command (seen on Drain/CTRL and Matmult/S3_LW). Tile
    occasionally aggregates several waits onto one instruction. Hoist all
    but the last wait onto preceding NoOps on the same engine."""
    n = [0]

    def fresh(base):
        n[0] += 1
        return f"{base}_wsplit{n[0]}"

    for fn in nc.m.functions:
        for blk in fn.blocks:
            out = []
            for inst in blk.instructions:
                si = getattr(inst, "sync_info", None)
                if si is not None and len(si.on_wait) > 1:
                    waits = list(si.on_wait)
                    for w in waits[:-1]:
                        d = mybir.InstNoOp(name=fresh(inst.name), ins=[], outs=[])
                        d.engine = inst.engine
                        d.sync_info = mybir.SyncInfo(on_wait=[w], on_update=[])
                        out.append(d)
                    inst.sync_info = mybir.SyncInfo(
                        on_wait=[waits[-1]], on_update=list(si.on_update)
                    )
                out.append(inst)
            blk.instructions[:] = out


def build_program(causal=True, split=True):
    """Build the per-core SPMD program.

    causal=True skips score/PV chunk blocks that a canonical causal mask
    fully masks out (host verifies the mask first), and only applies the
    mask DVE-add on diagonal blocks. causal=False handles any mask.
    """
    nc = bass.Bass(target_bir_lowering=False, num_swdge_queues=4)

    SQW = 512                  # matmul free width
    NSTW = S // SQW            # 4 strips per batch
    GC = 2                     # score chunks fused per PSUM group
    BLK = SQW // P             # 4 s_q blocks per strip

    hT = nc.dram_tensor("hT", [D, B * S], F32R, kind="ExternalInput")
    mT = nc.dram_tensor("mT", [S, S], F32, kind="ExternalInput")
    wqT = nc.dram_tensor("wqT", [D, DSH], F32R, kind="ExternalInput")
    wkT = nc.dram_tensor("wkT", [D, DSH], F32R, kind="ExternalInput")
    wvT = nc.dram_tensor("wvT", [D, DSH], F32R, kind="ExternalInput")
    woT = nc.dram_tensor("woT", [DSH, D], F32R, kind="ExternalInput")
    # rows: bq (pre-scaled), bk, bv, ones
    bqv = nc.dram_tensor("bqv", [4, SQW], F32R, kind="ExternalInput")
    ident_d = nc.dram_tensor("ident_d", [P, P], F32, kind="ExternalInput")
    outp = nc.dram_tensor("outp", [B * S, D], F32, kind="ExternalOutput")
    k_out = nc.dram_tensor("k_out", [B, HPC, S, HD], F32, kind="ExternalOutput")
    v_out = nc.dram_tensor("v_out", [B, HPC, S, HD], F32, kind="ExternalOutput")

    with tile.TileContext(nc) as tc:
        from contextlib import ExitStack

        with ExitStack() as top:
            constp = top.enter_context(tc.tile_pool(name="const", bufs=1))
            wproj = top.enter_context(tc.tile_pool(name="wproj", bufs=1))
            qkvp = top.enter_context(tc.tile_pool(name="qkv", bufs=1))
            streamp = top.enter_context(tc.tile_pool(name="stream", bufs=6))
            expsp = top.enter_context(tc.tile_pool(name="exps", bufs=1))
            maskp = top.enter_context(tc.tile_pool(name="mask", bufs=3))
            stagep = top.enter_context(tc.tile_pool(name="stage", bufs=3))

            identity = constp.tile([P, P], F32, name="identity", tag="identity")
            nc.sync.dma_start(identity[:], ident_d[:, :])

            wo_t = []
            for h in range(HPC):
                t = constp.tile([P, D], F32R, name=f"woT{h}", tag=f"woT{h}")
                nc.sync.dma_start(t[:], woT[h * P:(h + 1) * P, :])
                wo_t.append(t)

            wq_t, wk_t, wv_t = [], [], []
            for name, dram, lst in (("wq", wqT, wq_t), ("wk", wkT, wk_t),
                                    ("wv", wvT, wv_t)):
                for c in range(NCH):
                    t = wproj.tile([P, DSH], F32R, name=f"{name}{c}", tag=f"{name}{c}")
                    nc.sync.dma_start(t[:], dram[c * P:(c + 1) * P, :])
                    lst.append(t)

            bq_t = constp.tile([1, SQW], F32R, name="bq_t", tag="bq_t")
            bk_t = constp.tile([1, SQW], F32R, name="bk_t", tag="bk_t")
            bv_t = constp.tile([1, SQW], F32R, name="bv_t", tag="bv_t")
            ones_row = constp.tile([1, SQW], F32R, name="ones_row", tag="ones_row")
            for i, t in enumerate((bq_t, bk_t, bv_t, ones_row)):
                nc.sync.dma_start(t[:], bqv[i:i + 1, :])
            ones_col = constp.tile([P, 1], PV_DT, name="ones_col", tag="ones_col")
            nc.vector.memset(ones_col[:], 1.0)

            for b in range(B):
                qT = [qkvp.tile([P, S], F32R, name=f"qT{h}", tag=f"qT{h}")
                      for h in range(HPC)]
                kT = [qkvp.tile([P, S], F32R, name=f"kT{h}", tag=f"kT{h}")
                      for h in range(HPC)]
                vbf = [qkvp.tile([P, NCH * HD], PV_DT,
                                 name=f"vbf{h}", tag=f"vbf{h}")
                       for h in range(HPC)]
                ctxT = [qkvp.tile([P, S], F32R, name=f"ctxT{h}", tag=f"ctxT{h}")
                        for h in range(HPC)]

                # ---- phase 1: projections ----
                with tc.tile_pool(name="ps1", bufs=1, space="PSUM") as ps1, \
                     tc.tile_pool(name="ps1t", bufs=2, space="PSUM") as ps1t:
                    for st in range(NSTW):
                        s0 = st * SQW
                        psq = [ps1.tile([P, SQW], F32, name=f"psq{h}", tag=f"psq{h}")
                               for h in range(HPC)]
                        psk = [ps1.tile([P, SQW], F32, name=f"psk{h}", tag=f"psk{h}")
                               for h in range(HPC)]
                        psv = [ps1.tile([P, SQW], F32, name=f"psv{i}", tag=f"psv{i}")
                               for i in range(2)]
                        for c in range(NCH):
                            hc = streamp.tile([P, SQW], F32R,
                                              name="hchunk", tag="hchunk")
                            nc.sync.dma_start(
                                hc[:], hT[c * P:(c + 1) * P,
                                          b * S + s0: b * S + s0 + SQW])
                            for h in range(HPC):
                                hsl = slice(h * HD, (h + 1) * HD)
                                nc.tensor.matmul(psq[h][:], wq_t[c][:, hsl], hc[:],
                                                 start=(c == 0), stop=False)
                                nc.tensor.matmul(psk[h][:], wk_t[c][:, hsl], hc[:],
                                                 start=(c == 0), stop=False)
                            for s2 in range(BLK):
                                # one accumulation group per PSUM bank: start
                                # only on the first write into the tile
                                nc.tensor.matmul(
                                    psv[s2 // 2][:, (s2 % 2) * DSH:
                                                 (s2 % 2 + 1) * DSH],
                                    hc[:, s2 * P:(s2 + 1) * P], wv_t[c][:],
                                    start=(c == 0 and s2 % 2 == 0), stop=False)
                        for h in range(HPC):
                            hsl = slice(h * HD, (h + 1) * HD)
                            nc.tensor.matmul(psq[h][:], bq_t[:, hsl], ones_row[:],
                                             start=False, stop=True)
                            nc.tensor.matmul(psk[h][:], bk_t[:, hsl], ones_row[:],
                                             start=False, stop=True)
                            nc.scalar.copy(qT[h][:, s0:s0 + SQW], psq[h][:])
                            nc.scalar.copy(kT[h][:, s0:s0 + SQW], psk[h][:])
                        for s2 in range(BLK):
                            half = slice((s2 % 2) * DSH, (s2 % 2 + 1) * DSH)
                            nc.tensor.matmul(psv[s2 // 2][:, half],
                                             ones_row[:, :P], bv_t[:, :DSH],
                                             start=False, stop=(s2 % 2 == 1))
                        for s2 in range(BLK):
                            half = slice((s2 % 2) * DSH, (s2 % 2 + 1) * DSH)
                            sb = st * BLK + s2
                            vstage = stagep.tile([P, DSH], F32,
                                                 name="vstage", tag="vstage")
                            nc.vector.tensor_copy(vstage[:], psv[s2 // 2][:, half])
                            for h in range(HPC):
                                nc.sync.dma_start(
                                    v_out[b, h, sb * P:(sb + 1) * P, :],
                                    vstage[:, h * HD:(h + 1) * HD])
                                nc.vector.tensor_copy(
                                    vbf[h][:, sb * HD:(sb + 1) * HD],
                                    psv[s2 // 2][:, (s2 % 2) * DSH + h * HD:
                                                 (s2 % 2) * DSH + (h + 1) * HD])
                    # k natural layout for the k output
                    for h in range(HPC):
                        for sb in range(NSB):
                            pt = ps1t.tile([P, P], F32, name="pst", tag="pst")
                            nc.tensor.transpose(
                                pt[:], kT[h][:, sb * P:(sb + 1) * P].bitcast(F32),
                                identity[:])
                            kstage = stagep.tile([P, P], F32,
                                                 name="kstage", tag="kstage")
                            nc.scalar.copy(kstage[:], pt[:])
                            nc.sync.dma_start(k_out[b, h, sb * P:(sb + 1) * P, :],
                                              kstage[:])

                # ---- phase 2: attention ----
                # Chunk-level pipeline: each score group is exponentiated and
                # immediately folded into the ctx/denominator accumulators, so
                # the PE never waits a full strip for ACT. ctx for 4 s_q
                # blocks packs into one PSUM bank per head; denominators for
                # all (h, blk) pack into one bank, each via a single
                # accumulation group (start on first write, stop on last).
                with tc.tile_pool(name="ps2", bufs=2, space="PSUM") as ps2, \
                     tc.tile_pool(name="ps2c", bufs=1, space="PSUM") as ps2c, \
                     tc.tile_pool(name="ps2t", bufs=1, space="PSUM") as ps2t:
                    for st in range(NSTW):
                        s0 = st * SQW
                        if causal:
                            n_chunks = min(NCH, (st + 1) * (SQW // P))
                        else:
                            n_chunks = NCH
                        n_groups = (n_chunks + GC - 1) // GC
                        psC = [ps2c.tile([P, BLK * P], F32,
                                         name=f"psC{h}", tag=f"psC{h}")
                               for h in range(HPC)]
                        psD = ps2c.tile([P, HPC * BLK], F32,
                                        name="psD", tag="psD")
                        for g in range(n_groups):
                            cs = list(range(g * GC, min((g + 1) * GC, n_chunks)))
                            need_mask = (not causal) or any(
                                c * P + P - 1 >= s0 for c in cs)
                            if need_mask:
                                mt = maskp.tile([P, GC * SQW], F32,
                                                name="mask", tag="mask")
                                for i, c in enumerate(cs):
                                    nc.sync.dma_start(
                                        mt[:, i * SQW:(i + 1) * SQW],
                                        mT[c * P:(c + 1) * P, s0:s0 + SQW])
                            egs = []
                            for h in range(HPC):
                                ss = ps2.tile([P, GC * SQW], F32,
                                              name="psS", tag="psS")
                                for i, c in enumerate(cs):
                                    nc.tensor.matmul(
                                        ss[:, i * SQW:(i + 1) * SQW],
                                        kT[h][:, c * P:(c + 1) * P],
                                        qT[h][:, s0:s0 + SQW],
                                        start=True, stop=True)
                                w = len(cs) * SQW
                                if need_mask:
                                    nc.vector.tensor_add(ss[:, :w], ss[:, :w],
                                                         mt[:, :w])
                                eg = expsp.tile([P, GC * SQW], PV_DT,
                                                name=f"eg{h}", tag=f"eg{h}",
                                                bufs=3)
                                nc.scalar.activation(eg[:, :w], ss[:, :w], AF.Exp)
                                egs.append(eg)
                            for h in range(HPC):
                                for i, c in enumerate(cs):
                                    for blk in range(BLK):
                                        lhs = egs[h][:, i * SQW + blk * P:
                                                     i * SQW + (blk + 1) * P]
                                        nc.tensor.matmul(
                                            psC[h][:, blk * P:(blk + 1) * P],
                                            lhs, vbf[h][:, c * HD:(c + 1) * HD],
                                            start=(c == 0 and blk == 0),
                                            stop=(c == n_chunks - 1
                                                  and blk == BLK - 1))
                                        nc.tensor.matmul(
                                            psD[:, h * BLK + blk:
                                                h * BLK + blk + 1],
                                            lhs, ones_col[:],
                                            start=(c == 0 and blk == 0
                                                   and h == 0),
                                            stop=(c == n_chunks - 1
                                                  and blk == BLK - 1
                                                  and h == HPC - 1))
                        for h in range(HPC):
                            for blk in range(BLK):
                                recip = stagep.tile([P, 1], F32,
                                                    name="recip", tag="recip")
                                nc.vector.reciprocal(
                                    recip[:], psD[:, h * BLK + blk:
                                                  h * BLK + blk + 1])
                                cblk = stagep.tile([P, HD], F32R,
                                                   name="cblk", tag="cblk")
                                nc.vector.tensor_scalar_mul(
                                    cblk[:], psC[h][:, blk * P:(blk + 1) * P],
                                    recip[:])
                                pt = ps2t.tile([P, P], F32, name="pst2", tag="pst2")
                                nc.tensor.transpose(pt[:], cblk[:].bitcast(F32),
                                                    identity[:])
                                sqb = st * BLK + blk
                                nc.scalar.copy(
                                    ctxT[h][:, sqb * P:(sqb + 1) * P], pt[:])

                # ---- phase 3: out projection (partial) ----
                with tc.tile_pool(name="ps3", bufs=4, space="PSUM") as ps3:
                    for sb in range(NSB):
                        for mst in range(NSTW):
                            po = ps3.tile([P, SQW], F32, name="psO", tag="psO")
                            for h in range(HPC):
                                nc.tensor.matmul(
                                    po[:], ctxT[h][:, sb * P:(sb + 1) * P],
                                    wo_t[h][:, mst * SQW:(mst + 1) * SQW],
                                    start=(h == 0), stop=(h == HPC - 1))
                            ostage = stagep.tile([P, SQW], F32,
                                                 name="ostage", tag="ostage")
                            if (sb + mst) % 2 == 0:
                                nc.vector.tensor_copy(ostage[:], po[:])
                            else:
                                nc.scalar.copy(ostage[:], po[:])
                            nc.sync.dma_start(
                                outp[b * S + sb * P: b * S + (sb + 1) * P,
                                     mst * SQW:(mst + 1) * SQW], ostage[:])

    if split:
        _split_multiwait_insts(nc)
    return nc


_PROGRAMS = {}


def _get_program(causal):
    if causal not in _PROGRAMS:
        _PROGRAMS[causal] = build_program(causal=causal)
    return _PROGRAMS[causal]


def _is_causal_mask(mask2d):
    """True iff mask2d is the canonical additive causal mask: zeros on and
    below the diagonal, <= -1e9 strictly above."""
    iu = np.triu_indices(S, 1)
    if not (mask2d[iu] <= -1e9 + 1).all():
        return False
    il = np.tril_indices(S, 0)
    return (mask2d[il] == 0.0).all()


def _register_ntff_hook():
    """The image's antenv lacks axon_hooks; give bass_utils a working one."""
    try:
        import antenv.axon_hooks  # noqa: F401
        return
    except ImportError:
        pass
    try:
        import trn_agent_boot.trn_boot as tb
        hook = tb._ntff_profile_via_ctypes("/opt/axon/libaxon_pjrt.so")
    except Exception:
        hook = None
    mod = types.ModuleType("antenv.axon_hooks")
    mod.get_axon_ntff_profile_hook = lambda: hook
    mod.set_axon_ntff_profile_hook = lambda h: None
    sys.modules["antenv.axon_hooks"] = mod


def run_sharded(hidden_states, attn_mask, Wq, bq, Wk, bk, Wv, bv, Wo, bo,
                trace=False):
    """Shard inputs, run the 8-core SPMD kernel, gather. Returns
    ((out, k, v), BassKernelResults)."""
    _register_ntff_hook()
    f32 = np.float32
    hs = np.asarray(hidden_states, f32)
    mask = np.asarray(attn_mask, f32)
    Wq, bq = np.asarray(Wq, f32), np.asarray(bq, f32)
    Wk, bk = np.asarray(Wk, f32), np.asarray(bk, f32)
    Wv, bv = np.asarray(Wv, f32), np.asarray(bv, f32)
    Wo, bo = np.asarray(Wo, f32), np.asarray(bo, f32)

    sc = f32(1.0 / np.sqrt(HD))
    hT = np.ascontiguousarray(hs.reshape(B * S, D).T)
    causal = _is_causal_mask(mask[0, 0])
    mT = np.ascontiguousarray(np.maximum(mask[0, 0].T, MASK_CLAMP))
    ident = np.eye(P, dtype=f32)

    def brow(x):  # bias row padded to the 512-wide bqv layout
        row = np.zeros(512, f32)
        row[:DSH] = x
        return row

    in_maps = []
    for c in range(NCORES):
        r = slice(c * DSH, (c + 1) * DSH)
        in_maps.append({
            "hT": hT,
            "mT": mT,
            "wqT": np.ascontiguousarray((Wq[r, :] * sc).T),
            "wkT": np.ascontiguousarray(Wk[r, :].T),
            "wvT": np.ascontiguousarray(Wv[r, :].T),
            "woT": np.ascontiguousarray(Wo[:, r].T),
            "bqv": np.ascontiguousarray(
                np.stack([brow(bq[r] * sc), brow(bk[r]), brow(bv[r]),
                          np.ones(512, f32)])),
            "ident_d": ident,
        })

    nc = _get_program(causal)
    res = run_bass_kernel_spmd(nc, in_maps, core_ids=list(range(NCORES)),
                               trace=trace)

    out = np.zeros((B * S, D), f32)
    k = np.empty((B, H, S, HD), f32)
    v = np.empty((B, H, S, HD), f32)
    for c in range(NCORES):
        out += res.results[c]["outp"]
        k[:, c * HPC:(c + 1) * HPC] = res.results[c]["k_out"]
        v[:, c * HPC:(c + 1) * HPC] = res.results[c]["v_out"]
    out = (out + bo).reshape(B, S, D).astype(f32)
    return (out, k, v), res


def kernel(hidden_states, attn_mask, Wq, bq, Wk, bk, Wv, bv, Wo, bo):
    (out, k, v), _ = run_sharded(hidden_states, attn_mask,
                                 Wq, bq, Wk, bk, Wv, bv, Wo, bo)
    return out, k, v


# revision 13
# speedup vs baseline: 1.1372x; 1.1372x over previous
"""Multi-head attention (B=2, S=2048, D=2048, H=16) on 8 TRN2 NeuronCores.

Tensor-parallel over heads: each core computes 2 of 16 heads end to end
(q/k/v projections column-sharded, out_proj row-sharded) and writes a
partial output; the host sums the 8 partials and adds the output bias.

Per-core device pipeline (b = batch index, looped):
  phase 1: q^T,k^T ([HD, S] layout) and v ([S, HD] layout) via matmuls
           against host-pre-transposed hidden^T; biases are added with a
           K=1 matmul against a ones row. k is PE-transposed to [S, HD]
           for the k output.
  phase 2: scores^T[s_k, s_q] = k^T.T @ q^T (1/sqrt(HD) folded into Wq),
           + mask^T (DVE), exp (ACT, bf16 out), then ctx[s_q, :] =
           exp(S)^T.T @ [V | 1] -- the appended ones column yields the
           softmax denominator in the same accumulation. Divide by it
           (per-partition scalar), PE-transpose ctx to [HD, s_q].
  phase 3: out_partial[s, m] = sum_h ctx^T_h.T @ Wo^T_h.

Matmuls for projections/scores/out_proj run in float32r (single-pass
fp32 on the PE at bf16 rate); the PV matmul runs in bf16 on softmax
weights in [0,1]. All accumulation is fp32 in PSUM.
"""

import os
import sys
import types

for _p in ("/opt/trn_rl_repo", "/root/.axon_site/_ro/trn_rl_repo", "/root/.axon_site"):
    if os.path.isdir(_p) and _p not in sys.path:
        sys.path.append(_p)

import numpy as np

import concourse.bass as bass
import concourse.mybir as mybir
import concourse.tile as tile
from concourse.bass_utils import run_bass_kernel_spmd

# problem shape (hardcoded per contest contract)
B, S, D, H, HD = 2, 2048, 2048, 16, 128
NCORES = 8
HPC = H // NCORES          # heads per core = 2
DSH = HPC * HD             # per-core model-dim shard = 256
P = 128                    # partitions
NCH = D // P               # contraction chunks = 16
NSB = S // P               # 128-row blocks per batch = 16
SQ = 256                   # strip width (free dim of matmuls)
NST = S // SQ              # strips per batch = 8

F32 = mybir.dt.float32
F32R = mybir.dt.float32r   # projection/scores/out_proj matmul dtype
PV_DT = mybir.dt.bfloat16  # exp(S) and V-augmented dtypes for the PV matmul
AF = mybir.ActivationFunctionType

MASK_CLAMP = -60.0


def _split_multiwait_insts(nc):
    """This walrus build rejects any instruction carrying more than one
    sync-wait command (seen on Drain/CTRL and Matmult/S3_LW). Tile
    occasionally aggregates several waits onto one instruction. Hoist all
    but the last wait onto preceding NoOps on the same engine."""
    n = [0]

    def fresh(base):
        n[0] += 1
        return f"{base}_wsplit{n[0]}"

    for fn in nc.m.functions:
        for blk in fn.blocks:
            out = []
            for inst in blk.instructions:
                si = getattr(inst, "sync_info", None)
                if si is not None and len(si.on_wait) > 1:
                    waits = list(si.on_wait)
                    for w in waits[:-1]:
                        d = mybir.InstNoOp(name=fresh(inst.name), ins=[], outs=[])
                        d.engine = inst.engine
                        d.sync_info = mybir.SyncInfo(on_wait=[w], on_update=[])
                        out.append(d)
                    inst.sync_info = mybir.SyncInfo(
                        on_wait=[waits[-1]], on_update=list(si.on_update)
                    )
                out.append(inst)
            blk.instructions[:] = out


def build_program(causal=True, split=True):
    """Build the per-core SPMD program.

    causal=True skips score/PV chunk blocks that a canonical causal mask
    fully masks out (host verifies the mask first), and only applies the
    mask DVE-add on diagonal blocks. causal=False handles any mask.
    """
    nc = bass.Bass(target_bir_lowering=False, num_swdge_queues=4)

    SQW = 512                  # matmul free width
    NSTW = S // SQW            # 4 strips per batch
    GC = 2                     # score chunks fused per PSUM group
    BLK = SQW // P             # 4 s_q blocks per strip

    hT = nc.dram_tensor("hT", [D, B * S], F32R, kind="ExternalInput")
    mT = nc.dram_tensor("mT", [S, S], F32, kind="ExternalInput")
    wqT = nc.dram_tensor("wqT", [D, DSH], F32R, kind="ExternalInput")
    wkT = nc.dram_tensor("wkT", [D, DSH], F32R, kind="ExternalInput")
    wvT = nc.dram_tensor("wvT", [D, DSH], F32R, kind="ExternalInput")
    woT = nc.dram_tensor("woT", [DSH, D], F32R, kind="ExternalInput")
    # rows: bq (pre-scaled), bk, bv, ones
    bqv = nc.dram_tensor("bqv", [4, SQW], F32R, kind="ExternalInput")
    ident_d = nc.dram_tensor("ident_d", [P, P], F32, kind="ExternalInput")
    outp = nc.dram_tensor("outp", [B * S, D], F32, kind="ExternalOutput")
    k_out = nc.dram_tensor("k_out", [B, HPC, S, HD], F32, kind="ExternalOutput")
    v_out = nc.dram_tensor("v_out", [B, HPC, S, HD], F32, kind="ExternalOutput")

    with tile.TileContext(nc) as tc:
        from contextlib import ExitStack

        with ExitStack() as top:
            constp = top.enter_context(tc.tile_pool(name="const", bufs=1))
            wproj = top.enter_context(tc.tile_pool(name="wproj", bufs=1))
            qkvp = top.enter_context(tc.tile_pool(name="qkv", bufs=1))
            streamp = top.enter_context(tc.tile_pool(name="stream", bufs=6))
            expsp = top.enter_context(tc.tile_pool(name="exps", bufs=1))
            maskp = top.enter_context(tc.tile_pool(name="mask", bufs=3))
            stagep = top.enter_context(tc.tile_pool(name="stage", bufs=3))

            identity = constp.tile([P, P], F32, name="identity", tag="identity")
            nc.sync.dma_start(identity[:], ident_d[:, :])

            wo_t = [constp.tile([P, D], F32R, name=f"woT{h}", tag=f"woT{h}")
                    for h in range(HPC)]  # loaded later, first needed in out_proj

            wq_t, wk_t, wv_t = [], [], []
            for c in range(NCH):
                for name, dram, lst in (("wq", wqT, wq_t), ("wk", wkT, wk_t),
                                        ("wv", wvT, wv_t)):
                    t = wproj.tile([P, DSH], F32R, name=f"{name}{c}", tag=f"{name}{c}")
                    nc.sync.dma_start(t[:], dram[c * P:(c + 1) * P, :])
                    lst.append(t)

            bq_t = constp.tile([1, SQW], F32R, name="bq_t", tag="bq_t")
            bk_t = constp.tile([1, SQW], F32R, name="bk_t", tag="bk_t")
            bv_t = constp.tile([1, SQW], F32R, name="bv_t", tag="bv_t")
            ones_row = constp.tile([1, SQW], F32R, name="ones_row", tag="ones_row")
            for i, t in enumerate((bq_t, bk_t, bv_t, ones_row)):
                nc.sync.dma_start(t[:], bqv[i:i + 1, :])
            ones_col = constp.tile([P, 1], PV_DT, name="ones_col", tag="ones_col")
            nc.vector.memset(ones_col[:], 1.0)

            for b in range(B):
                qT = [qkvp.tile([P, S], F32R, name=f"qT{h}", tag=f"qT{h}")
                      for h in range(HPC)]
                kT = [qkvp.tile([P, S], F32R, name=f"kT{h}", tag=f"kT{h}")
                      for h in range(HPC)]
                vbf = [qkvp.tile([P, NCH * HD], PV_DT,
                                 name=f"vbf{h}", tag=f"vbf{h}")
                       for h in range(HPC)]
                ctxT = [qkvp.tile([P, S], F32R, name=f"ctxT{h}", tag=f"ctxT{h}")
                        for h in range(HPC)]

                # ---- phase 1: projections ----
                with tc.tile_pool(name="ps1", bufs=1, space="PSUM") as ps1, \
                     tc.tile_pool(name="ps1t", bufs=2, space="PSUM") as ps1t:
                    for st in range(NSTW):
                        s0 = st * SQW
                        psq = [ps1.tile([P, SQW], F32, name=f"psq{h}", tag=f"psq{h}")
                               for h in range(HPC)]
                        psk = [ps1.tile([P, SQW], F32, name=f"psk{h}", tag=f"psk{h}")
                               for h in range(HPC)]
                        psv = [ps1.tile([P, SQW], F32, name=f"psv{i}", tag=f"psv{i}")
                               for i in range(2)]
                        for c in range(NCH):
                            hc = streamp.tile([P, SQW], F32R,
                                              name="hchunk", tag="hchunk")
                            nc.sync.dma_start(
                                hc[:], hT[c * P:(c + 1) * P,
                                          b * S + s0: b * S + s0 + SQW])
                            for h in range(HPC):
                                hsl = slice(h * HD, (h + 1) * HD)
                                nc.tensor.matmul(psq[h][:], wq_t[c][:, hsl], hc[:],
                                                 start=(c == 0), stop=False)
                                nc.tensor.matmul(psk[h][:], wk_t[c][:, hsl], hc[:],
                                                 start=(c == 0), stop=False)
                            for s2 in range(BLK):
                                # one accumulation group per PSUM bank: start
                                # only on the first write into the tile
                                nc.tensor.matmul(
                                    psv[s2 // 2][:, (s2 % 2) * DSH:
                                                 (s2 % 2 + 1) * DSH],
                                    hc[:, s2 * P:(s2 + 1) * P], wv_t[c][:],
                                    start=(c == 0 and s2 % 2 == 0), stop=False)
                        for h in range(HPC):
                            hsl = slice(h * HD, (h + 1) * HD)
                            nc.tensor.matmul(psq[h][:], bq_t[:, hsl], ones_row[:],
                                             start=False, stop=True)
                            nc.tensor.matmul(psk[h][:], bk_t[:, hsl], ones_row[:],
                                             start=False, stop=True)
                            nc.scalar.copy(qT[h][:, s0:s0 + SQW], psq[h][:])
                            nc.scalar.copy(kT[h][:, s0:s0 + SQW], psk[h][:])
                        for s2 in range(BLK):
                            half = slice((s2 % 2) * DSH, (s2 % 2 + 1) * DSH)
                            nc.tensor.matmul(psv[s2 // 2][:, half],
                                             ones_row[:, :P], bv_t[:, :DSH],
                                             start=False, stop=(s2 % 2 == 1))
                        for s2 in range(BLK):
                            half = slice((s2 % 2) * DSH, (s2 % 2 + 1) * DSH)
                            sb = st * BLK + s2
                            vstage = stagep.tile([P, DSH], F32,
                                                 name="vstage", tag="vstage")
                            nc.vector.tensor_copy(vstage[:], psv[s2 // 2][:, half])
                            for h in range(HPC):
                                nc.sync.dma_start(
                                    v_out[b, h, sb * P:(sb + 1) * P, :],
                                    vstage[:, h * HD:(h + 1) * HD])
                                nc.vector.tensor_copy(
                                    vbf[h][:, sb * HD:(sb + 1) * HD],
                                    psv[s2 // 2][:, (s2 % 2) * DSH + h * HD:
                                                 (s2 % 2) * DSH + (h + 1) * HD])
                    # k natural layout for the k output
                    for h in range(HPC):
                        for sb in range(NSB):
                            pt = ps1t.tile([P, P], F32, name="pst", tag="pst")
                            nc.tensor.transpose(
                                pt[:], kT[h][:, sb * P:(sb + 1) * P].bitcast(F32),
                                identity[:])
                            kstage = stagep.tile([P, P], F32,
                                                 name="kstage", tag="kstage")
                            nc.scalar.copy(kstage[:], pt[:])
                            nc.sync.dma_start(k_out[b, h, sb * P:(sb + 1) * P, :],
                                              kstage[:])

                # ---- phase 2+3: attention fused with out projection ----
                # Chunk-level pipeline: each score chunk is exponentiated and
                # immediately folded into the ctx/denominator accumulators, so
                # the PE never waits a full strip for ACT. ctx for 4 s_q
                # blocks packs into one PSUM bank per head; denominators for
                # all (h, blk) pack into one bank, each via a single
                # accumulation group (start on first write, stop on last).
                # out_proj runs per strip as soon as ctx^T is ready, keeping
                # one PSUM pool alive through the whole region.
                if b == 0:
                    for h in range(HPC):
                        nc.sync.dma_start(wo_t[h][:], woT[h * P:(h + 1) * P, :])
                with tc.tile_pool(name="ps2", bufs=2, space="PSUM") as ps2, \
                     tc.tile_pool(name="ps2c", bufs=1, space="PSUM") as ps2c, \
                     tc.tile_pool(name="ps2t", bufs=1, space="PSUM") as ps2t, \
                     tc.tile_pool(name="ps3", bufs=2, space="PSUM") as ps3:
                    for st in range(NSTW):
                        s0 = st * SQW
                        if causal:
                            n_chunks = min(NCH, (st + 1) * (SQW // P))
                        else:
                            n_chunks = NCH
                        psC = [ps2c.tile([P, BLK * P], F32,
                                         name=f"psC{h}", tag=f"psC{h}")
                               for h in range(HPC)]
                        psD = ps2c.tile([P, HPC * BLK], F32,
                                        name="psD", tag="psD")
                        for c in range(n_chunks):
                            need_mask = (not causal) or (c * P + P - 1 >= s0)
                            if need_mask:
                                mt = maskp.tile([P, SQW], F32,
                                                name="mask", tag="mask")
                                nc.sync.dma_start(
                                    mt[:], mT[c * P:(c + 1) * P, s0:s0 + SQW])
                            for h in range(HPC):
                                ss = ps2.tile([P, SQW], F32,
                                              name="psS", tag="psS")
                                nc.tensor.matmul(
                                    ss[:], kT[h][:, c * P:(c + 1) * P],
                                    qT[h][:, s0:s0 + SQW],
                                    start=True, stop=True)
                                if need_mask:
                                    nc.vector.tensor_add(ss[:], ss[:], mt[:])
                                eg = expsp.tile([P, SQW], PV_DT,
                                                name=f"eg{h}", tag=f"eg{h}",
                                                bufs=3)
                                nc.scalar.activation(eg[:], ss[:], AF.Exp)
                                for blk in range(BLK):
                                    lhs = eg[:, blk * P:(blk + 1) * P]
                                    nc.tensor.matmul(
                                        psC[h][:, blk * P:(blk + 1) * P],
                                        lhs, vbf[h][:, c * HD:(c + 1) * HD],
                                        start=(c == 0 and blk == 0),
                                        stop=(c == n_chunks - 1
                                              and blk == BLK - 1))
                                    nc.tensor.matmul(
                                        psD[:, h * BLK + blk:
                                            h * BLK + blk + 1],
                                        lhs, ones_col[:],
                                        start=(c == 0 and blk == 0 and h == 0),
                                        stop=(c == n_chunks - 1
                                              and blk == BLK - 1
                                              and h == HPC - 1))
                        for h in range(HPC):
                            for blk in range(BLK):
                                recip = stagep.tile([P, 1], F32,
                                                    name="recip", tag="recip")
                                nc.vector.reciprocal(
                                    recip[:], psD[:, h * BLK + blk:
                                                  h * BLK + blk + 1])
                                cblk = stagep.tile([P, HD], F32R,
                                                   name="cblk", tag="cblk")
                                nc.vector.tensor_scalar_mul(
                                    cblk[:], psC[h][:, blk * P:(blk + 1) * P],
                                    recip[:])
                                pt = ps2t.tile([P, P], F32, name="pst2", tag="pst2")
                                nc.tensor.transpose(pt[:], cblk[:].bitcast(F32),
                                                    identity[:])
                                sqb = st * BLK + blk
                                nc.scalar.copy(
                                    ctxT[h][:, sqb * P:(sqb + 1) * P], pt[:])
                        for blk in range(BLK):
                            sb = st * BLK + blk
                            for mst in range(NSTW):
                                po = ps3.tile([P, SQW], F32, name="psO", tag="psO")
                                for h in range(HPC):
                                    nc.tensor.matmul(
                                        po[:], ctxT[h][:, sb * P:(sb + 1) * P],
                                        wo_t[h][:, mst * SQW:(mst + 1) * SQW],
                                        start=(h == 0), stop=(h == HPC - 1))
                                ostage = stagep.tile([P, SQW], F32,
                                                     name="ostage", tag="ostage")
                                if (sb + mst) % 2 == 0:
                                    nc.vector.tensor_copy(ostage[:], po[:])
                                else:
                                    nc.scalar.copy(ostage[:], po[:])
                                nc.sync.dma_start(
                                    outp[b * S + sb * P: b * S + (sb + 1) * P,
                                         mst * SQW:(mst + 1) * SQW], ostage[:])

    if split:
        _split_multiwait_insts(nc)
    return nc


_PROGRAMS = {}


def _get_program(causal):
    if causal not in _PROGRAMS:
        _PROGRAMS[causal] = build_program(causal=causal)
    return _PROGRAMS[causal]


def _is_causal_mask(mask2d):
    """True iff mask2d is the canonical additive causal mask: zeros on and
    below the diagonal, <= -1e9 strictly above."""
    iu = np.triu_indices(S, 1)
    if not (mask2d[iu] <= -1e9 + 1).all():
        return False
    il = np.tril_indices(S, 0)
    return (mask2d[il] == 0.0).all()


def _register_ntff_hook():
    """The image's antenv lacks axon_hooks; give bass_utils a working one."""
    try:
        import antenv.axon_hooks  # noqa: F401
        return
    except ImportError:
        pass
    try:
        import trn_agent_boot.trn_boot as tb
        hook = tb._ntff_profile_via_ctypes("/opt/axon/libaxon_pjrt.so")
    except Exception:
        hook = None
    mod = types.ModuleType("antenv.axon_hooks")
    mod.get_axon_ntff_profile_hook = lambda: hook
    mod.set_axon_ntff_profile_hook = lambda h: None
    sys.modules["antenv.axon_hooks"] = mod


def run_sharded(hidden_states, attn_mask, Wq, bq, Wk, bk, Wv, bv, Wo, bo,
                trace=False):
    """Shard inputs, run the 8-core SPMD kernel, gather. Returns
    ((out, k, v), BassKernelResults)."""
    _register_ntff_hook()
    f32 = np.float32
    hs = np.asarray(hidden_states, f32)
    mask = np.asarray(attn_mask, f32)
    Wq, bq = np.asarray(Wq, f32), np.asarray(bq, f32)
    Wk, bk = np.asarray(Wk, f32), np.asarray(bk, f32)
    Wv, bv = np.asarray(Wv, f32), np.asarray(bv, f32)
    Wo, bo = np.asarray(Wo, f32), np.asarray(bo, f32)

    sc = f32(1.0 / np.sqrt(HD))
    hT = np.ascontiguousarray(hs.reshape(B * S, D).T)
    causal = _is_causal_mask(mask[0, 0])
    mT = np.ascontiguousarray(np.maximum(mask[0, 0].T, MASK_CLAMP))
    ident = np.eye(P, dtype=f32)

    def brow(x):  # bias row padded to the 512-wide bqv layout
        row = np.zeros(512, f32)
        row[:DSH] = x
        return row

    in_maps = []
    for c in range(NCORES):
        r = slice(c * DSH, (c + 1) * DSH)
        in_maps.append({
            "hT": hT,
            "mT": mT,
            "wqT": np.ascontiguousarray((Wq[r, :] * sc).T),
            "wkT": np.ascontiguousarray(Wk[r, :].T),
            "wvT": np.ascontiguousarray(Wv[r, :].T),
            "woT": np.ascontiguousarray(Wo[:, r].T),
            "bqv": np.ascontiguousarray(
                np.stack([brow(bq[r] * sc), brow(bk[r]), brow(bv[r]),
                          np.ones(512, f32)])),
            "ident_d": ident,
        })

    nc = _get_program(causal)
    res = run_bass_kernel_spmd(nc, in_maps, core_ids=list(range(NCORES)),
                               trace=trace)

    out = np.zeros((B * S, D), f32)
    k = np.empty((B, H, S, HD), f32)
    v = np.empty((B, H, S, HD), f32)
    for c in range(NCORES):
        out += res.results[c]["outp"]
        k[:, c * HPC:(c + 1) * HPC] = res.results[c]["k_out"]
        v[:, c * HPC:(c + 1) * HPC] = res.results[c]["v_out"]
    out = (out + bo).reshape(B, S, D).astype(f32)
    return (out, k, v), res


def kernel(hidden_states, attn_mask, Wq, bq, Wk, bk, Wv, bv, Wo, bo):
    (out, k, v), _ = run_sharded(hidden_states, attn_mask,
                                 Wq, bq, Wk, bk, Wv, bv, Wo, bo)
    return out, k, v
